# revision 25
# baseline (speedup 1.0000x reference)
"""Trainium2 Bass kernel for a dense transformer block (nn_Block_90185723281472).

Strategy
--------
- Data-parallel over batch: B=8 batch elements -> 8 NeuronCores, one full
  transformer block per core, zero collectives.
- Feature-transposed space throughout: activations live in SBUF as
  [C(=768, partition dim in 128-chunks), N(=1025 tokens, free dim)].
  Residual stream is bf16 end-to-end; matmuls bf16 with fp32 PSUM.
- Attention: the two heads of a pair are computed as adjacent K=64 matmuls
  into one 2-bank PSUM tile; the PE row-tiles them concurrently and ONE
  scalar-engine exp covers both.  PV for chunk jc-1 is emitted after scores
  for jc (software pipelining).
- Softmax normalization: the ones-column denominator rows of all 12 heads are
  gathered into a [12, w] tile, inverted with ONE DVE reciprocal per token
  tile (lanes are parallel, so [12,w] costs the same as [1,w]), then rank-1
  broadcast + in-place multiply.  The scalar engine runs *only* Exp during
  attention - no activation-table switches.
- MLP/proj work for token tile i-1 is interleaved, ~2 matmuls per jc step,
  into attention for tile i: the PE stays dense (HAM stays un-throttled) and
  the exp latency is fully hidden.  fc1 evicts pre-activation values (bias
  folded in as a rank-1 matmul); gelu runs as one batched in-place pass per
  token tile so it costs only 2 activation-table switches.
- V is produced directly in natural [token, feature] layout by using LN1
  token-chunks as the matmul stationary operand (no PE transposes).
- LayerNorm: PE ones-matmul stats, single Rsqrt, rank-1 broadcast of mu*rs
  and rs, bf16 apply ops.
"""

import os
import sys

for _p in ("/opt/trn_rl_repo", "/root/.axon_site/_ro/trn_rl_repo"):
    if os.path.isdir(_p) and _p not in sys.path:
        sys.path.insert(0, _p)

from collections import deque
from contextlib import ExitStack

import ml_dtypes
import numpy as np

import concourse.bass as bass
import concourse.tile as tile
from concourse import bacc
from concourse import mybir
from concourse.bass_utils import run_bass_kernel_spmd

F32 = mybir.dt.float32
BF16 = mybir.dt.bfloat16
AF = mybir.ActivationFunctionType
ALU = mybir.AluOpType
AX = mybir.AxisListType

B = 8
C = 768                 # embed dim
NT = 1025               # tokens
NJP = 1152              # tokens padded to 9*128 (key/j axis only)
NCC = C // 128          # 6 feature chunks
H, HD, NHP = 12, 64, 6  # heads, head dim, head pairs
HID = 4 * C             # 3072
NHC = HID // 128        # 24 hidden chunks
NJC = NJP // 128        # 9 j-chunks
ITS = [(0, 384), (384, 384), (768, 257)]   # (offset, width) token tiles
SW = 512                # PSUM sub-tile stride (one bank of fp32)
EPS = 1e-5
SCALE = float(HD) ** -0.5


def _ln_tile_gen(nc, lsb, lst, psp, srcs, w_pp, b_pp, dst, io, w,
                 ones_c, ones_r, eps_sb):
    """LayerNorm of one token tile in transposed space; srcs are bf16.

    Generator: yields at PE-work boundaries so the emission driver can
    interleave it with attention.  PSUM comes from `psp` tag "acc" (2 bufs:
    mu/s2 then bmr/brs rotate through the same two slots).
    """
    mu_ps = psp.tile([1, SW], F32, tag="acc", name="mu_ps")
    s2_ps = psp.tile([1, SW], F32, tag="acc", name="s2_ps")
    for cc in range(NCC):
        sq = lsb.tile([128, 384], BF16, tag="sq", name="sq")
        nc.vector.tensor_mul(sq[:, :w], srcs[cc], srcs[cc])
        nc.tensor.matmul(mu_ps[:, :w], ones_c, srcs[cc],
                         start=(cc == 0), stop=(cc == NCC - 1))
        nc.tensor.matmul(s2_ps[:, :w], ones_c, sq[:, :w],
                         start=(cc == 0), stop=(cc == NCC - 1))
        yield
    muf = lst.tile([1, 384], F32, tag="st", name="muf")
    nc.vector.tensor_scalar_mul(muf[:, :w], mu_ps[:, :w], 1.0 / C)
    ex2 = lst.tile([1, 384], F32, tag="st", name="ex2")
    nc.vector.tensor_scalar_mul(ex2[:, :w], s2_ps[:, :w], 1.0 / C)
    musq = lst.tile([1, 384], F32, tag="st", name="musq")
    nc.scalar.activation(musq[:, :w], muf[:, :w], AF.Square)
    var = lst.tile([1, 384], F32, tag="st", name="var")
    nc.vector.tensor_sub(var[:, :w], ex2[:, :w], musq[:, :w])
    sd = lst.tile([1, 384], F32, tag="st", name="sd")
    nc.scalar.activation(sd[:, :w], var[:, :w], AF.Sqrt, bias=eps_sb[0:1, :])
    rsd = lst.tile([1, 384], F32, tag="st", name="rsd")
    nc.vector.reciprocal(rsd[:, :w], sd[:, :w])
    rsb = lst.tile([1, 384], BF16, tag="stb", name="rsb", bufs=2)
    nc.vector.tensor_copy(rsb[:, :w], rsd[:, :w])
    murs = lst.tile([1, 384], BF16, tag="stb", name="murs", bufs=2)
    with nc.allow_low_precision(reason="ln mean*rs as bf16 rank-1 operand"):
        nc.vector.tensor_tensor(murs[:, :w], muf[:, :w], rsd[:, :w],
                                op=ALU.mult)
    yield
    bmr_ps = psp.tile([128, SW], F32, tag="acc", name="bmr_ps")
    nc.tensor.matmul(bmr_ps[:, :w], ones_r, murs[:, :w])
    brs_ps = psp.tile([128, SW], F32, tag="acc", name="brs_ps")
    nc.tensor.matmul(brs_ps[:, :w], ones_r, rsb[:, :w])
    bmr = lst.tile([128, 384], BF16, tag="bcb", name="bmr", bufs=2)
    nc.vector.tensor_copy(bmr[:, :w], bmr_ps[:, :w])
    brs = lst.tile([128, 384], BF16, tag="bcb", name="brs", bufs=2)
    nc.vector.tensor_copy(brs[:, :w], brs_ps[:, :w])
    yield
    for cc in range(NCC):
        t1 = lsb.tile([128, 384], BF16, tag="t1", name="t1")
        nc.vector.tensor_tensor(t1[:, :w], srcs[cc], brs[:, :w], op=ALU.mult)
        nc.vector.tensor_sub(t1[:, :w], t1[:, :w], bmr[:, :w])
        nc.vector.tensor_scalar(dst[:, cc, io:io + w], t1[:, :w],
                                w_pp[:, cc:cc + 1], b_pp[:, cc:cc + 1],
                                op0=ALU.mult, op1=ALU.add)
        yield


def _build_program():
    nc = bacc.Bacc("TRN2", target_bir_lowering=False)

    xT = nc.dram_tensor("xT", [C, NT], F32, kind="ExternalInput").ap()
    wqkv = nc.dram_tensor("wqkv", [C, 3 * C], BF16, kind="ExternalInput").ap()
    wproj = nc.dram_tensor("wproj", [C, C], BF16, kind="ExternalInput").ap()
    wfc1 = nc.dram_tensor("wfc1", [C, HID], BF16, kind="ExternalInput").ap()
    wfc2 = nc.dram_tensor("wfc2", [HID, C], BF16, kind="ExternalInput").ap()
    wca1 = nc.dram_tensor("wca1", [C, 192], F32, kind="ExternalInput").ap()
    wca2 = nc.dram_tensor("wca2", [192, C], F32, kind="ExternalInput").ap()
    ln1w = nc.dram_tensor("ln1w", [C], F32, kind="ExternalInput").ap()
    ln1b = nc.dram_tensor("ln1b", [C], F32, kind="ExternalInput").ap()
    ln2w = nc.dram_tensor("ln2w", [C], F32, kind="ExternalInput").ap()
    ln2b = nc.dram_tensor("ln2b", [C], F32, kind="ExternalInput").ap()
    bproj = nc.dram_tensor("bproj", [C], F32, kind="ExternalInput").ap()
    bfc1 = nc.dram_tensor("bfc1", [HID], F32, kind="ExternalInput").ap()
    bfc2 = nc.dram_tensor("bfc2", [C], F32, kind="ExternalInput").ap()
    outT = nc.dram_tensor("outT", [C, NT], F32, kind="ExternalOutput").ap()

    with tile.TileContext(nc) as tc, ExitStack() as ctx:
        # ---------------- constants ----------------
        cpool = ctx.enter_context(tc.tile_pool(name="consts", bufs=1))
        ones_c = cpool.tile([128, 1], BF16, tag="ones_c", name="ones_c")
        nc.vector.memset(ones_c, 1.0)
        ones_r = cpool.tile([1, 128], BF16, tag="ones_r", name="ones_r")
        nc.vector.memset(ones_r, 1.0)
        ones64 = cpool.tile([1, 64], BF16, tag="ones64", name="ones64")
        nc.vector.memset(ones64, 1.0)
        ones_rw = cpool.tile([1, SW], BF16, tag="ones_rw", name="ones_rw")
        nc.vector.memset(ones_rw, 1.0)
        mask8 = cpool.tile([128, 1], F32, tag="mask8", name="mask8")
        nc.vector.memset(mask8, -40.0)
        nc.vector.memset(mask8[0:1, :], 0.0)
        eps_sb = cpool.tile([128, 1], F32, tag="eps_sb", name="eps_sb")
        nc.vector.memset(eps_sb, EPS)

        def load_pcvec(ap, nchunk, name):
            t = cpool.tile([128, nchunk], F32, tag=name, name=name)
            nc.sync.dma_start(t, ap.rearrange("(cc p) -> p cc", p=128))
            return t

        ln1w_pp = load_pcvec(ln1w, NCC, "ln1w_pp")
        ln1b_pp = load_pcvec(ln1b, NCC, "ln1b_pp")
        ln2w_pp = load_pcvec(ln2w, NCC, "ln2w_pp")
        ln2b_pp = load_pcvec(ln2b, NCC, "ln2b_pp")
        bfc1_row = cpool.tile([1, HID], BF16, tag="bfc1_row", name="bfc1_row")
        bfc2_row = cpool.tile([1, C], BF16, tag="bfc2_row", name="bfc2_row")
        bproj_row = cpool.tile([1, C], BF16, tag="bproj_row", name="bproj_row")
        with tc.tile_pool(name="stage", bufs=1) as stg:
            bfc1_rf = stg.tile([1, HID], F32, tag="s1", name="bfc1_rf")
            nc.sync.dma_start(bfc1_rf, bfc1.rearrange("(o n) -> o n", o=1))
            nc.vector.tensor_copy(bfc1_row, bfc1_rf)
            bfc2_rf = stg.tile([1, C], F32, tag="s2", name="bfc2_rf")
            nc.sync.dma_start(bfc2_rf, bfc2.rearrange("(o n) -> o n", o=1))
            nc.vector.tensor_copy(bfc2_row, bfc2_rf)
            bproj_rf = stg.tile([1, C], F32, tag="s3", name="bproj_rf")
            nc.sync.dma_start(bproj_rf, bproj.rearrange("(o n) -> o n", o=1))
            nc.vector.tensor_copy(bproj_row, bproj_rf)
        # resident weights: all of W_v (natural-V matmul rhs) and W_proj
        wv_sb = cpool.tile([128, NCC, C], BF16, tag="wv_sb", name="wv_sb")
        nc.sync.dma_start(wv_sb,
                          wqkv[:, 2 * C:3 * C].rearrange("(cc p) n -> p cc n",
                                                         p=128))
        wproj_sb = cpool.tile([128, NCC, C], BF16, tag="wproj_sb",
                              name="wproj_sb")
        nc.sync.dma_start(wproj_sb, wproj.rearrange("(cc p) n -> p cc n",
                                                    p=128))

        # ctx-wide pools
        wcol = ctx.enter_context(tc.tile_pool(name="wcol", bufs=3))
        acc = ctx.enter_context(tc.tile_pool(name="acc", bufs=2, space="PSUM"))

        # ---------------- stack tiles (A/B era) ----------------
        # pre-CA pooling accumulators (written by the last fc2 units)
        capx = ctx.enter_context(tc.tile_pool(name="capx", bufs=1))
        mx = capx.tile([128, NCC], F32, tag="mx", name="ca_mx")
        av = capx.tile([128, NCC], F32, tag="av", name="ca_av")

        qT, free_qT = tc.tile([128, NCC, NT], BF16, name="qT")
        kT, free_kT = tc.tile([128, NCC, NJP], BF16, name="kT")
        vnat, free_vnat = tc.tile([128, NJC, H, HD + 1], BF16, name="vnat")
        nc.vector.memset(kT[:, :, NT:NJP], 0.0)
        nc.vector.memset(vnat[:, :, :, HD:HD + 1], 1.0)
        h_bf, free_h_bf = tc.tile([128, NCC, NJP], BF16, name="h_bf")
        nc.vector.memset(h_bf[:, :, NT:NJP], 0.0)

        # ---------------- phase A: load x, cast, LN1 -> h_bf ----------------
        with (
            tc.tile_pool(name="xstage", bufs=2) as xsp,
            tc.tile_pool(name="ln1_sb", bufs=3) as l1sb,
            tc.tile_pool(name="ln1_st", bufs=4) as l1st,
        ):
            for io, w in ITS:
                xf = xsp.tile([128, NCC, 384], F32, tag="xf", name="xf")
                nc.sync.dma_start(
                    xf[:, :, :w],
                    xT.rearrange("(cc p) n -> p cc n", p=128)[:, :, io:io + w])
                xb = xsp.tile([128, NCC, 384], BF16, tag="xb", name="xb")
                for cc in range(NCC):
                    nc.vector.tensor_copy(xb[:, cc, :w], xf[:, cc, :w])
                srcs = [xb[:, cc, :w] for cc in range(NCC)]
                for _ in _ln_tile_gen(nc, l1sb, l1st, acc, srcs, ln1w_pp,
                                      ln1b_pp, h_bf, io, w, ones_c, ones_r,
                                      eps_sb):
                    pass

        # --- interleaved-work emission machinery (build-time only) ---
        units = deque()
        cur = [None]
        done_units = set()

        def advance(n):
            while n > 0:
                if cur[0] is None:
                    if not units:
                        return
                    nm, fac = units.popleft()
                    cur[0] = (nm, fac())
                try:
                    next(cur[0][1])
                    n -= 1
                except StopIteration:
                    done_units.add(cur[0][0])
                    cur[0] = None

        def drain_until(name):
            while name not in done_units:
                assert cur[0] is not None or units, f"unit {name} missing"
                advance(1)

        # ---------------- phase B: qk (transposed) + v (natural) ----------
        def qk_unit(m):
            def g():
                for oc in (m, m + NCC):
                    wt = wcol.tile([128, NCC, 128], BF16, tag="wcol",
                                   name="wt_qk")
                    nc.sync.dma_start(
                        wt, wqkv[:, oc * 128:(oc + 1) * 128]
                        .rearrange("(cc p) n -> p cc n", p=128))
                    for io, w in ITS:
                        ps = acc.tile([128, SW], F32, tag="acc", name="qk_ps")
                        for cc in range(NCC):
                            nc.tensor.matmul(ps[:, :w], wt[:, cc, :],
                                             h_bf[:, cc, io:io + w],
                                             start=(cc == 0),
                                             stop=(cc == NCC - 1))
                            if cc % 2 == 1:
                                yield
                        if oc < NCC:
                            nc.vector.tensor_copy(qT[:, oc, io:io + w],
                                                  ps[:, :w])
                        else:
                            nc.vector.tensor_copy(kT[:, oc - NCC, io:io + w],
                                                  ps[:, :w])
                        yield
            return g

        for m in range(2):
            for _ in qk_unit(m)():
                pass
        for jb in range(NJC):
            for fh in range(2):
                vp = acc.tile([128, NCC, HD], F32, tag="acc", name="v_ps")
                for cc in range(NCC):
                    nc.tensor.matmul(
                        vp, h_bf[:, cc, jb * 128:(jb + 1) * 128],
                        wv_sb[:, cc, fh * 384:(fh + 1) * 384],
                        start=(cc == 0), stop=(cc == NCC - 1))
                nc.vector.tensor_copy(
                    vnat[:, jb, 6 * fh:6 * fh + 6, 0:HD], vp)
        for m in range(2, NHP):
            units.append((f"qk{m}", qk_unit(m)))

        # ------------- phases C-G: attention with interleaved MLP ----------
        with tc.tile_pool(name="bigp", bufs=1) as bigp:
            oT = bigp.tile([128, NCC, NT], BF16, tag="oT", name="oT")
            x1 = bigp.tile([128, NCC, NT], BF16, tag="x1", name="x1")
            h2_bf = bigp.tile([128, NCC, NT], BF16, tag="h2", name="h2_bf")
            mlpT = bigp.tile([128, NHC, NT], BF16, tag="mlp", name="mlpT")

            with (
                tc.tile_pool(name="spool", bufs=2, space="PSUM") as spool,
                tc.tile_pool(name="opool", bufs=2, space="PSUM") as opool,
                tc.tile_pool(name="ptp", bufs=3) as ptp,
                tc.tile_pool(name="dallp", bufs=2) as dallp,
                tc.tile_pool(name="xinp", bufs=2) as xinp,
                tc.tile_pool(name="w2p", bufs=2) as w2p,
                tc.tile_pool(name="ln2_sb", bufs=2) as l2sb,
                tc.tile_pool(name="ln2_st", bufs=4) as l2st,
            ):
                def proj_unit(iti, p):
                    io, w = ITS[iti]

                    def g():
                        for oc in (2 * p, 2 * p + 1):
                            xin = xinp.tile([128, 384], F32, tag="xin",
                                            name="xin")
                            nc.sync.dma_start(
                                xin[:, :w],
                                xT[oc * 128:(oc + 1) * 128, io:io + w])
                            ps = acc.tile([128, SW], F32, tag="acc",
                                          name="proj_ps")
                            for cc in range(NCC):
                                nc.tensor.matmul(
                                    ps[:, :w],
                                    wproj_sb[:, cc, oc * 128:(oc + 1) * 128],
                                    oT[:, cc, io:io + w],
                                    start=(cc == 0), stop=False)
                                if cc % 2 == 1:
                                    yield
                            nc.tensor.matmul(
                                ps[:, :w],
                                bproj_row[0:1, oc * 128:(oc + 1) * 128],
                                ones_rw[:, :w], start=False, stop=True)
                            nc.vector.tensor_tensor(x1[:, oc, io:io + w],
                                                    ps[:, :w],
                                                    xin[:, :w],
                                                    op=ALU.add)
                            yield
                    return g

                def ln2_unit(iti):
                    io, w = ITS[iti]

                    def g():
                        srcs = [x1[:, cc, io:io + w] for cc in range(NCC)]
                        yield from _ln_tile_gen(nc, l2sb, l2st, acc, srcs,
                                                ln2w_pp, ln2b_pp, h2_bf, io,
                                                w, ones_c, ones_r, eps_sb)
                    return g

                def fc1_unit(iti, q):
                    io, w = ITS[iti]

                    def g():
                        for hc in range(4 * q, 4 * q + 4):
                            wt = wcol.tile([128, NCC, 128], BF16, tag="wcol",
                                           name="wt_f1")
                            nc.sync.dma_start(
                                wt, wfc1[:, hc * 128:(hc + 1) * 128]
                                .rearrange("(cc p) n -> p cc n", p=128))
                            ps = acc.tile([128, SW], F32, tag="acc",
                                          name="fc1_ps")
                            for cc in range(NCC):
                                nc.tensor.matmul(ps[:, :w], wt[:, cc, :],
                                                 h2_bf[:, cc, io:io + w],
                                                 start=(cc == 0), stop=False)
                                if cc % 2 == 1:
                                    yield
                            nc.tensor.matmul(
                                ps[:, :w],
                                bfc1_row[0:1, hc * 128:(hc + 1) * 128],
                                ones_rw[:, :w], start=False, stop=True)
                            nc.vector.tensor_copy(mlpT[:, hc, io:io + w],
                                                  ps[:, :w])
                            yield
                    return g

                def gelu_unit(iti):
                    io, w = ITS[iti]

                    def g():
                        for gq in range(6):
                            nc.scalar.activation(
                                mlpT[:, 4 * gq:4 * gq + 4, io:io + w],
                                mlpT[:, 4 * gq:4 * gq + 4, io:io + w],
                                AF.Gelu)
                        yield
                    return g

                def fc2_unit(iti, oc):
                    io, w = ITS[iti]

                    def g():
                        wt2 = w2p.tile([128, NHC, 128], BF16, tag="w2",
                                       name="wt_fc2")
                        nc.sync.dma_start(
                            wt2, wfc2[:, oc * 128:(oc + 1) * 128]
                            .rearrange("(hc p) n -> p hc n", p=128))
                        ps = acc.tile([128, SW], F32, tag="acc",
                                      name="fc2_ps")
                        for hc in range(NHC):
                            nc.tensor.matmul(ps[:, :w], wt2[:, hc, :],
                                             mlpT[:, hc, io:io + w],
                                             start=(hc == 0), stop=False)
                            if hc % 2 == 1:
                                yield
                        nc.tensor.matmul(
                            ps[:, :w],
                            bfc2_row[0:1, oc * 128:(oc + 1) * 128],
                            ones_rw[:, :w], start=False, stop=True)
                        nc.vector.tensor_add(x1[:, oc, io:io + w], ps[:, :w],
                                             x1[:, oc, io:io + w])
                        if iti == len(ITS) - 1:
                            nc.vector.reduce_max(mx[:, oc:oc + 1],
                                                 x1[:, oc, 1:NT], axis=AX.X)
                            nc.vector.reduce_sum(av[:, oc:oc + 1],
                                                 x1[:, oc, 1:NT], axis=AX.X)
                        yield
                    return g

                def attn_block(hp, io, w, dqs):
                    o_ps = [opool.tile([HD + 1, SW], F32, tag="o",
                                       name="o_ps") for _ in range(2)]
                    prev_pt = None
                    for jc in range(NJC):
                        s = spool.tile([128, 2, SW], F32, tag="s", name="s_ps")
                        for h2 in range(2):
                            pb = 64 * h2
                            nc.tensor.matmul(
                                s[:, h2, :w],
                                kT[pb:pb + 64, hp, jc * 128:(jc + 1) * 128],
                                qT[pb:pb + 64, hp, io:io + w])
                        pt = ptp.tile([128, 2, 384], BF16, tag="pt", name="pt")
                        nc.scalar.activation(
                            pt[:, :, :w], s[:, :, :w], AF.Exp,
                            bias=(mask8 if jc == NJC - 1 else 0.0),
                            scale=SCALE)
                        advance(2)
                        if prev_pt is not None:
                            for h2 in range(2):
                                nc.tensor.matmul(
                                    o_ps[h2][:, :w],
                                    vnat[:, jc - 1, 2 * hp + h2, :],
                                    prev_pt[:, h2, :w],
                                    start=(jc == 1), stop=False)
                        prev_pt = pt
                    for h2 in range(2):
                        nc.tensor.matmul(
                            o_ps[h2][:, :w],
                            vnat[:, NJC - 1, 2 * hp + h2, :],
                            prev_pt[:, h2, :w], start=False, stop=True)
                    for h2 in range(2):
                        h = 2 * hp + h2
                        q, r = divmod(h, 4)
                        nc.vector.tensor_copy(oT[64 * h2:64 * h2 + 64, hp,
                                                 io:io + w],
                                              o_ps[h2][0:HD, :w])
                        with nc.allow_low_precision(
                                reason="softmax denominators to bf16"):
                            nc.vector.tensor_copy(
                                dqs[q][32 * r:32 * r + 1, :w],
                                o_ps[h2][HD:HD + 1, :w])
                    advance(6)

                def denorm(io, w, dqs):
                    rdqs = []
                    for q in range(3):
                        rdq = dallp.tile([97, 384], BF16, tag="rdq",
                                         name="rdq", bufs=3)
                        with nc.allow_low_precision(
                                reason="softmax denominators to bf16"):
                            nc.vector.reciprocal(rdq[:, :w], dqs[q][:, :w])
                        rdqs.append(rdq)
                    for hp in range(NHP):
                        for h2 in range(2):
                            h = 2 * hp + h2
                            q, r = divmod(h, 4)
                            r0 = dallp.tile([1, 384], BF16, tag="r0",
                                            name="r0", bufs=2)
                            nc.vector.tensor_copy(
                                r0[:, :w], rdqs[q][32 * r:32 * r + 1, :w])
                            rb = spool.tile([64, SW], F32, tag="s",
                                            name="rb_ps")
                            nc.tensor.matmul(rb[:, :w], ones64, r0[:, :w])
                            nc.vector.tensor_tensor(
                                oT[64 * h2:64 * h2 + 64, hp, io:io + w],
                                oT[64 * h2:64 * h2 + 64, hp, io:io + w],
                                rb[0:64, :w], op=ALU.mult)

                for iti, (io, w) in enumerate(ITS):
                    dqs = []
                    for q in range(3):
                        dq = dallp.tile([97, 384], BF16, tag="dq", name="dq",
                                        bufs=3)
                        nc.vector.memset(dq, 1.0)
                        dqs.append(dq)
                    for hp in range(NHP):
                        if iti == 0 and hp >= 2:
                            drain_until(f"qk{hp}")
                        attn_block(hp, io, w, dqs)
                    denorm(io, w, dqs)
                    for p in range(3):
                        units.append((f"proj{iti}_{p}", proj_unit(iti, p)))
                    units.append((f"ln2_{iti}", ln2_unit(iti)))
                    for q in range(6):
                        units.append((f"fc1_{iti}_{q}", fc1_unit(iti, q)))
                    units.append((f"gelu_{iti}", gelu_unit(iti)))
                    for oc in range(NCC):
                        units.append((f"fc2_{iti}_{oc}", fc2_unit(iti, oc)))
                while units or cur[0] is not None:
                    advance(64)

            # ------------- phase H: channel attention + writeout -----------
            with (
                tc.tile_pool(name="cap", bufs=1) as cap,
                tc.tile_pool(name="caps", bufs=2, space="PSUM") as caps,
                tc.tile_pool(name="outp", bufs=2) as outp,
            ):
                wca1_sb = cap.tile([128, NCC, 192], F32, tag="wca1_sb",
                                   name="wca1_sb")
                nc.sync.dma_start(wca1_sb,
                                  wca1.rearrange("(cc p) n -> p cc n", p=128))
                wca2a_sb = cap.tile([128, C], F32, tag="wca2a",
                                    name="wca2a_sb")
                nc.sync.dma_start(wca2a_sb, wca2[0:128, :])
                wca2b_sb = cap.tile([64, C], F32, tag="wca2b",
                                    name="wca2b_sb")
                nc.sync.dma_start(wca2b_sb, wca2[128:192, :])
                nc.vector.tensor_scalar_mul(av, av, 1.0 / (NT - 1))
                relus = []
                for bi, pool_t in enumerate((mx, av)):
                    ga = caps.tile([128, 1], F32, tag="g1a", name="ca_ga")
                    gb = caps.tile([64, 1], F32, tag="g1b", name="ca_gb")
                    for cc in range(NCC):
                        nc.tensor.matmul(ga, wca1_sb[:, cc, 0:128],
                                         pool_t[:, cc:cc + 1],
                                         start=(cc == 0), stop=(cc == NCC - 1))
                        nc.tensor.matmul(gb, wca1_sb[:, cc, 128:192],
                                         pool_t[:, cc:cc + 1],
                                         start=(cc == 0), stop=(cc == NCC - 1))
                    ra = cap.tile([128, 1], F32, tag=f"ra{bi}", name="ca_ra")
                    nc.vector.tensor_relu(ra, ga)
                    rb2 = cap.tile([64, 1], F32, tag=f"rb{bi}", name="ca_rb")
                    nc.vector.tensor_relu(rb2, gb)
                    relus.append((ra, rb2))
                for oc in range(NCC):
                    gt = caps.tile([128, 1], F32, tag="gt", name="ca_gt")
                    k = 0
                    for ra, rb2 in relus:
                        nc.tensor.matmul(gt,
                                         wca2a_sb[:, oc * 128:(oc + 1) * 128],
                                         ra, start=(k == 0), stop=False)
                        k += 1
                        nc.tensor.matmul(gt,
                                         wca2b_sb[:, oc * 128:(oc + 1) * 128],
                                         rb2, start=False, stop=(k == 3))
                        k += 1
                    gs = cap.tile([128, 1], F32, tag="gs", name="ca_gs")
                    nc.scalar.activation(gs, gt, AF.Sigmoid)
                    nc.vector.tensor_scalar_add(gs, gs, 1.0)
                    ob = outp.tile([128, NT], F32, tag="ob", name="ob")
                    nc.vector.tensor_copy(ob[:, 0:1], x1[:, oc, 0:1])
                    nc.vector.tensor_scalar(ob[:, 1:NT], x1[:, oc, 1:NT],
                                            gs, None, op0=ALU.mult)
                    nc.sync.dma_start(outT[oc * 128:(oc + 1) * 128, :], ob)

        for _f in (free_h_bf, free_vnat, free_kT, free_qT):
            _f()

    nc.compile()
    return nc


_CACHE = {}


def _get_program():
    if "nc" not in _CACHE:
        _CACHE["nc"] = _build_program()
    return _CACHE["nc"]


def _make_in_maps(inputs):
    bf = ml_dtypes.bfloat16
    f32 = np.float32

    def as_np(a, dt=f32):
        return np.ascontiguousarray(np.asarray(a, dtype=f32).astype(dt))

    base = {
        "wqkv": as_np(inputs["qkv_w"], bf),
        "wproj": as_np(inputs["proj_w"], bf),
        "wfc1": as_np(inputs["fc1_w"], bf),
        "wfc2": as_np(inputs["fc2_w"], bf),
        "wca1": as_np(inputs["ca1_w"]),
        "wca2": as_np(inputs["ca2_w"]),
        "ln1w": as_np(inputs["ln1_w"]),
        "ln1b": as_np(inputs["ln1_b"]),
        "ln2w": as_np(inputs["ln2_w"]),
        "ln2b": as_np(inputs["ln2_b"]),
        "bproj": as_np(inputs["proj_b"]),
        "bfc1": as_np(inputs["fc1_b"]),
        "bfc2": as_np(inputs["fc2_b"]),
    }
    x = np.asarray(inputs["x"], dtype=f32)
    in_maps = []
    for b in range(B):
        m = dict(base)
        m["xT"] = np.ascontiguousarray(x[b].T)
        in_maps.append(m)
    return in_maps


def kernel(**inputs) -> np.ndarray:
    nc = _get_program()
    in_maps = _make_in_maps(inputs)
    res = run_bass_kernel_spmd(nc, in_maps, list(range(B)))
    out = np.stack([np.asarray(res.results[b]["outT"]).T for b in range(B)])
    return np.ascontiguousarray(out.astype(np.float32))


if __name__ == "__main__":
    nc = _get_program()
    n_inst = sum(len(bb.instructions) for bb in nc.main_func.blocks)
    print(f"program built: {n_inst} instructions")
